# revision 1
# baseline (speedup 1.0000x reference)
"""Trainium2 kernel for out = A @ W2 @ B.T with banded Gaussian W2.

Math: W2 = W1*W1 where W1[i,j] = exp(-(i-j)^2/(2*8^2)) truncated below 1e-10.
W1 > eps only for |i-j| <= 54, so in 128-blocks W2 is block-tridiagonal AND
translation-invariant: only three distinct 128x128 blocks exist (diag D0,
super-diag U = W2[j-1,j], sub-diag L = W2[j+1,j] = U.T).

Strategy (data-parallel over A's rows, 8 cores, no collectives):
  - host: transpose A and B once, build the three W2 blocks.
  - each core gets A.T slab [4096, 1024], full B.T, the W2 pack.
  - phase 1 (once): TT = W2 @ A.T  (= (A_slab @ W2).T), banded block-tridiag
    matmuls over the narrow A-slab; TT [4096, 1024] stays resident in SBUF.
  - phase 2 (per 512-col chunk nu): out[:, nu] = TT.T @ B.T[:, nu], with all
    8 PSUM banks accumulating the 8 m-tiles while B.T streams through once.
  - all matmuls run as float32r (FP22 multiply, FP32 accumulate) -> 1 cyc/row.
"""

import numpy as np

import concourse.bass as bass
import concourse.mybir as mybir
from concourse import bacc
from concourse.bass_utils import run_bass_kernel_spmd
from concourse.tile import TileContext

P = 128          # partition / block size
N = 4096         # inner dims (A cols, B rows/cols)
M_FULL = 8192    # A rows
NCORES = 8
MS = M_FULL // NCORES   # 1024 rows of A per core
NK = N // P      # 32 contraction blocks
NM = MS // P     # 8 m-tiles per core
CW = 512         # output column chunk width (= 1 PSUM bank of fp32)
NCH = N // CW    # 8 chunks
NH = MS // CW    # 2 column-halves of the A.T slab in phase 1

SIGMA = 8.0
TRUNC_EPS = 1e-10

_COMPILED = {}


def _w2_block(dist):
    """W2 entries for a matrix of absolute diagonal distances."""
    d = dist.astype(np.float32)
    w1 = np.exp(-(d * d) / np.float32(2.0 * SIGMA * SIGMA)).astype(np.float32)
    w1 = np.where(w1 > np.float32(TRUNC_EPS), w1, np.float32(0.0)).astype(np.float32)
    return (w1 * w1).astype(np.float32)


def _build_w2_pack():
    a = np.arange(P)[:, None]
    b = np.arange(P)[None, :]
    d0 = _w2_block(np.abs(a - b))          # W2[j, j]
    u = _w2_block(np.abs(a - b - P))       # W2[j-1, j]
    l = _w2_block(np.abs(P + a - b))       # W2[j+1, j]
    return np.ascontiguousarray(np.concatenate([d0, u, l], axis=1))  # [128, 384]


def _build_program(reps=1):
    """Build + compile the Bass program (one NEFF, run SPMD on 8 cores).

    reps>1 repeats the whole computation serially inside the NEFF (same
    result; used only for timing calibration).
    """
    nc = bacc.Bacc("TRN2", target_bir_lowering=False, debug=False)
    f32 = mybir.dt.float32
    f32r = mybir.dt.float32r

    at_dram = nc.dram_tensor("at", [N, MS], f32r, kind="ExternalInput").ap()
    bt_dram = nc.dram_tensor("bt", [N, N], f32r, kind="ExternalInput").ap()
    w2_dram = nc.dram_tensor("w2", [P, 3 * P], f32r, kind="ExternalInput").ap()
    out_dram = nc.dram_tensor("out", [MS, N], f32, kind="ExternalOutput").ap()

    with TileContext(nc) as tc:
        with (
            tc.tile_pool(name="const", bufs=1) as const_pool,
            tc.tile_pool(name="atp", bufs=6) as at_pool,
            tc.tile_pool(name="ttp", bufs=1) as tt_pool,
            tc.tile_pool(name="btp", bufs=4) as bt_pool,
            tc.tile_pool(name="obp", bufs=4) as ob_pool,
            tc.tile_pool(name="psp", bufs=8, space="PSUM") as ps_pool,
        ):
            w2_sb = const_pool.tile([P, 3 * P], f32r, tag="w2", name="w2_sb")
            nc.sync.dma_start(w2_sb, w2_dram)
            # lhsT for contribution d: W2[j+d, j]
            w2_lhsT = {
                0: w2_sb[:, 0:P],
                -1: w2_sb[:, P:2 * P],
                1: w2_sb[:, 2 * P:3 * P],
            }

            for rep in range(reps):
                # --- phase 1: TT = W2 @ A.T ([4096, 1024], resident in SBUF)
                at_tiles = [None] * NK

                def get_at(k, rep=rep, at_tiles_=None):
                    if at_tiles[k] is None:
                        at_t = at_pool.tile([P, MS], f32r, tag="at",
                                            name=f"at_sb_{rep}_{k}")
                        nc.sync.dma_start(at_t, at_dram[k * P:(k + 1) * P, :])
                        at_tiles[k] = at_t
                    return at_tiles[k]

                tt_tiles = []
                for j in range(NK):
                    tt_t = tt_pool.tile([P, MS], f32r, tag=f"tt{j}",
                                        name=f"tt_sb_{rep}_{j}")
                    dlist = [d for d in (-1, 0, 1) if 0 <= j + d < NK]
                    for h in range(NH):
                        hs = bass.ts(h, CW)
                        ps_t = ps_pool.tile([P, CW], f32, tag="ps",
                                            name=f"ps_t_{rep}_{j}_{h}")
                        for i, d in enumerate(dlist):
                            nc.tensor.matmul(
                                ps_t,
                                lhsT=w2_lhsT[d],
                                rhs=get_at(j + d)[:, hs],
                                start=(i == 0),
                                stop=(i == len(dlist) - 1),
                            )
                        nc.vector.tensor_copy(tt_t[:, hs], ps_t)
                    tt_tiles.append(tt_t)

                # --- phase 2: out = TT.T @ B.T, streamed in 512-col chunks
                for nu in range(NCH):
                    cs = bass.ts(nu, CW)
                    ps_o = [
                        ps_pool.tile([P, CW], f32, tag="ps",
                                     name=f"ps_o_{rep}_{nu}_{m}")
                        for m in range(NM)
                    ]
                    for k in range(NK):
                        bt_t = bt_pool.tile([P, CW], f32r, tag="bt",
                                            name=f"bt_sb_{rep}_{nu}_{k}")
                        nc.sync.dma_start(
                            bt_t, bt_dram[k * P:(k + 1) * P, cs]
                        )
                        for m in range(NM):
                            nc.tensor.matmul(
                                ps_o[m],
                                lhsT=tt_tiles[k][:, m * P:(m + 1) * P],
                                rhs=bt_t,
                                start=(k == 0),
                                stop=(k == NK - 1),
                            )
                    for m in range(NM):
                        ob_t = ob_pool.tile([P, CW], f32, tag="ob",
                                            name=f"ob_sb_{rep}_{nu}_{m}")
                        if m % 2 == 0:
                            nc.vector.tensor_copy(ob_t, ps_o[m])
                        else:
                            nc.scalar.copy(ob_t, ps_o[m])
                        nc.sync.dma_start(
                            out_dram[m * P:(m + 1) * P, cs], ob_t
                        )

    nc.compile()
    return nc


def _get_program():
    if "nc" not in _COMPILED:
        _COMPILED["nc"] = _build_program()
    return _COMPILED["nc"]


def kernel(A, B):
    A = np.ascontiguousarray(np.asarray(A, dtype=np.float32))
    B = np.ascontiguousarray(np.asarray(B, dtype=np.float32))
    assert A.shape == (M_FULL, N), A.shape
    assert B.shape == (N, N), B.shape

    a_t = np.ascontiguousarray(A.T)          # [4096, 8192]
    b_t = np.ascontiguousarray(B.T)          # [4096, 4096]
    w2_pack = _build_w2_pack()               # [128, 384]

    in_maps = [
        {
            "at": np.ascontiguousarray(a_t[:, c * MS:(c + 1) * MS]),
            "bt": b_t,
            "w2": w2_pack,
        }
        for c in range(NCORES)
    ]

    nc = _get_program()
    res = run_bass_kernel_spmd(nc, in_maps, core_ids=list(range(NCORES)))
    return np.concatenate(
        [res.results[c]["out"] for c in range(NCORES)], axis=0
    ).astype(np.float32)



# revision 3
# speedup vs baseline: 1.1622x; 1.1622x over previous
"""Trainium2 kernel for out = A @ W2 @ B.T with banded Gaussian W2.

Math: W2 = W1*W1 where W1[i,j] = exp(-(i-j)^2/(2*8^2)) truncated below 1e-10.
W1 > eps only for |i-j| <= 54, so in 128-blocks W2 is block-tridiagonal AND
translation-invariant: only three distinct 128x128 blocks exist (diag D0,
super-diag U = W2[j-1,j], sub-diag L = W2[j+1,j] = U.T).

Strategy (data-parallel over A's rows, 8 cores, no collectives):
  - host: transpose A and B once, cast to bf16, build the three W2 blocks.
  - each core gets A.T slab [4096, 1024], full B.T, the W2 pack (all bf16).
  - phase 1 (once): TT = W2 @ A.T  (= (A_slab @ W2).T), banded block-tridiag
    matmuls over the narrow A-slab; TT [4096, 1024] bf16 stays in SBUF.
  - phase 2 (per 512-col chunk nu): out[:, nu] = TT.T @ B.T[:, nu], with all
    8 PSUM banks accumulating the 8 m-tiles while B.T streams through once.
    B.T is loaded one whole 512-col chunk (4 MB) per DMA, double-buffered;
    output staged in one wide SBUF tile and stored with one 2 MB DMA/chunk.
  - bf16 matmuls run 1 cyc/row with FP32 PSUM accumulation; FWL makes the
    per-matmul LDWEIGHTS (128x128 bf16) cheap enough to hide.
"""

import numpy as np

import concourse.bass as bass
import concourse.mybir as mybir
from concourse import bacc
from concourse.bass_utils import run_bass_kernel_spmd
from concourse.tile import TileContext

P = 128          # partition / block size
N = 4096         # inner dims (A cols, B rows/cols)
M_FULL = 8192    # A rows
NCORES = 8
MS = M_FULL // NCORES   # 1024 rows of A per core
NK = N // P      # 32 contraction blocks
NM = MS // P     # 8 m-tiles per core
CW = 512         # output column chunk width (= 1 PSUM bank of fp32)
NCH = N // CW    # 8 chunks
NH = MS // CW    # 2 column-halves of the A.T slab in phase 1

SIGMA = 8.0
TRUNC_EPS = 1e-10

_COMPILED = {}


def _w2_block(dist):
    """W2 entries for a matrix of absolute diagonal distances."""
    d = dist.astype(np.float32)
    w1 = np.exp(-(d * d) / np.float32(2.0 * SIGMA * SIGMA)).astype(np.float32)
    w1 = np.where(w1 > np.float32(TRUNC_EPS), w1, np.float32(0.0)).astype(np.float32)
    return (w1 * w1).astype(np.float32)


def _build_w2_pack():
    a = np.arange(P)[:, None]
    b = np.arange(P)[None, :]
    d0 = _w2_block(np.abs(a - b))          # W2[j, j]
    u = _w2_block(np.abs(a - b - P))       # W2[j-1, j]
    l = _w2_block(np.abs(P + a - b))       # W2[j+1, j]
    return np.ascontiguousarray(np.concatenate([d0, u, l], axis=1))  # [128, 384]


def _build_program(reps=1):
    """Build + compile the Bass program (one NEFF, run SPMD on 8 cores).

    reps>1 repeats the whole computation serially inside the NEFF (same
    result; used only for timing calibration).
    """
    nc = bacc.Bacc("TRN2", target_bir_lowering=False, debug=False)
    f32 = mybir.dt.float32
    bf16 = mybir.dt.bfloat16

    at_dram = nc.dram_tensor("at", [N, MS], bf16, kind="ExternalInput").ap()
    bt_dram = nc.dram_tensor("bt", [N, N], bf16, kind="ExternalInput").ap()
    w2_dram = nc.dram_tensor("w2", [P, 3 * P], bf16, kind="ExternalInput").ap()
    out_dram = nc.dram_tensor("out", [MS, N], f32, kind="ExternalOutput").ap()

    # [128, 32, 4096]: partition p of k-block row k*128+p
    bt3 = bt_dram.rearrange("(k p) c -> p k c", p=P)
    # [128, 8, 4096]: partition p of m-tile row m*128+p
    out3 = out_dram.rearrange("(m p) c -> p m c", p=P)

    with TileContext(nc) as tc:
        with (
            tc.tile_pool(name="const", bufs=1) as const_pool,
            tc.tile_pool(name="atp", bufs=6) as at_pool,
            tc.tile_pool(name="ttp", bufs=1) as tt_pool,
            tc.tile_pool(name="btc", bufs=2) as btc_pool,
            tc.tile_pool(name="obw", bufs=2) as obw_pool,
            tc.tile_pool(name="psp", bufs=8, space="PSUM") as ps_pool,
        ):
            w2_sb = const_pool.tile([P, 3 * P], bf16, tag="w2", name="w2_sb")
            nc.sync.dma_start(w2_sb, w2_dram)
            # lhsT for contribution d: W2[j+d, j]
            w2_lhsT = {
                0: w2_sb[:, 0:P],
                -1: w2_sb[:, P:2 * P],
                1: w2_sb[:, 2 * P:3 * P],
            }

            for rep in range(reps):
                # --- phase 1: TT = W2 @ A.T ([4096, 1024] bf16, SBUF-resident)
                at_tiles = [None] * NK

                def get_at(k, rep=rep):
                    if at_tiles[k] is None:
                        at_t = at_pool.tile([P, MS], bf16, tag="at",
                                            name=f"at_sb_{rep}_{k}")
                        nc.sync.dma_start(at_t, at_dram[k * P:(k + 1) * P, :])
                        at_tiles[k] = at_t
                    return at_tiles[k]

                tt_tiles = []
                for j in range(NK):
                    tt_t = tt_pool.tile([P, MS], bf16, tag=f"tt{j}",
                                        name=f"tt_sb_{rep}_{j}")
                    dlist = [d for d in (-1, 0, 1) if 0 <= j + d < NK]
                    for h in range(NH):
                        hs = bass.ts(h, CW)
                        ps_t = ps_pool.tile([P, CW], f32, tag="ps",
                                            name=f"ps_t_{rep}_{j}_{h}")
                        for i, d in enumerate(dlist):
                            nc.tensor.matmul(
                                ps_t,
                                lhsT=w2_lhsT[d],
                                rhs=get_at(j + d)[:, hs],
                                start=(i == 0),
                                stop=(i == len(dlist) - 1),
                            )
                        nc.vector.tensor_copy(tt_t[:, hs], ps_t)
                    tt_tiles.append(tt_t)

                # --- phase 2: out = TT.T @ B.T, streamed in 512-col chunks
                for nu in range(NCH):
                    cs = bass.ts(nu, CW)
                    btc_t = btc_pool.tile([P, NK * CW], bf16, tag="btc",
                                          name=f"btc_sb_{rep}_{nu}")
                    nc.sync.dma_start(
                        btc_t.rearrange("p (k c) -> p k c", c=CW),
                        bt3[:, :, cs],
                    )
                    ps_o = [
                        ps_pool.tile([P, CW], f32, tag="ps",
                                     name=f"ps_o_{rep}_{nu}_{m}")
                        for m in range(NM)
                    ]
                    for k in range(NK):
                        ks = bass.ts(k, CW)
                        for m in range(NM):
                            nc.tensor.matmul(
                                ps_o[m],
                                lhsT=tt_tiles[k][:, m * P:(m + 1) * P],
                                rhs=btc_t[:, ks],
                                start=(k == 0),
                                stop=(k == NK - 1),
                            )
                    obw_t = obw_pool.tile([P, NM * CW], f32, tag="obw",
                                          name=f"obw_sb_{rep}_{nu}")
                    for m in range(NM):
                        if m % 2 == 0:
                            nc.vector.tensor_copy(
                                obw_t[:, m * CW:(m + 1) * CW], ps_o[m])
                        else:
                            nc.scalar.copy(
                                obw_t[:, m * CW:(m + 1) * CW], ps_o[m])
                    nc.sync.dma_start(
                        out3[:, :, cs],
                        obw_t.rearrange("p (m c) -> p m c", c=CW),
                    )

    nc.compile()
    return nc


def _get_program():
    if "nc" not in _COMPILED:
        _COMPILED["nc"] = _build_program()
    return _COMPILED["nc"]


def _build_in_maps(A, B):
    import ml_dtypes

    A = np.asarray(A, dtype=np.float32)
    B = np.asarray(B, dtype=np.float32)
    assert A.shape == (M_FULL, N), A.shape
    assert B.shape == (N, N), B.shape

    a_t = np.ascontiguousarray(A.T.astype(ml_dtypes.bfloat16))  # [4096, 8192]
    b_t = np.ascontiguousarray(B.T.astype(ml_dtypes.bfloat16))  # [4096, 4096]
    w2_pack = _build_w2_pack().astype(ml_dtypes.bfloat16)       # [128, 384]

    return [
        {
            "at": np.ascontiguousarray(a_t[:, c * MS:(c + 1) * MS]),
            "bt": b_t,
            "w2": w2_pack,
        }
        for c in range(NCORES)
    ]


def kernel(A, B):
    in_maps = _build_in_maps(A, B)
    nc = _get_program()
    res = run_bass_kernel_spmd(nc, in_maps, core_ids=list(range(NCORES)))
    return np.concatenate(
        [res.results[c]["out"] for c in range(NCORES)], axis=0
    ).astype(np.float32)


# revision 6
# speedup vs baseline: 1.7281x; 1.4869x over previous
"""Trainium2 kernel for out = A @ W2 @ B.T with banded Gaussian W2.

Math: W2 = W1*W1, W1[i,j] = exp(-(i-j)^2/(2*8^2)) truncated below 1e-10, so
W2 = exp(-(i-j)^2/64) on a |i-j| <= 54 band. W2 is symmetric banded Toeplitz.

Embed W2 in the 4096-circulant C with the same band symbol:
    C = W2 + E,  E = two 54x54 corner triangles (the wrap-around band).
C diagonalizes in the real DFT basis: C = Q_full diag(lam) Q_full.T with
lam(f) ~ sqrt(64*pi) * exp(-16 * (2*pi*f/N)^2): only the lowest ~384
frequencies matter. Keeping r = 768 basis vectors (f=0, cos/sin f=1..383,
cos f=384) gives truncation error ~1.5e-3 (validated numerically), so

    out = (A @ Q) @ (B @ Q*lam).T - A @ E @ B.T

cuts the big-GEMM FLOPs ~2.6x vs A @ W2 @ B.T done densely over 4096.

Distribution (8 cores, one AllGather):
  - core c holds: A.T slab [4096, 1024] (its 1024 A-rows), B.T column slice
    [4096, 512], Q and Q*lam [4096, 768], corner packs (all bf16).
  - stage H (sharded): HT_loc = (Q*lam).T @ B.T[:, c-slice]  [768, 512]
    -> DRAM AllGather -> HT full [768, 4096] available to every core.
  - stage G (data-parallel): GT = Q.T @ A.T_slab  [768, 1024], plus the two
    corner factors GTc = (-E_corner).T @ A.T_corner [128, 1024] each.
  - final: out_slab = GT.T @ HT + GTc.T @ B.T_corner, streamed per 512-col
    chunk with all 8 PSUM banks accumulating the 8 m-tiles.
  - all matmuls bf16 (1 cyc/row, FP32 PSUM accumulate); out stored bf16 and
    upcast on host (adds ~2e-3 rounding, total rel err ~5e-3 < 2e-2).
"""

import numpy as np

import concourse.bass as bass
import concourse.mybir as mybir
from concourse import bacc
from concourse.bass_utils import run_bass_kernel_spmd
from concourse.tile import TileContext
from concourse.tile_rust import add_dep_helper

P = 128          # partition / block size
N = 4096         # inner dims (A cols, B rows/cols)
M_FULL = 8192    # A rows
NCORES = 8
MS = M_FULL // NCORES   # 1024 rows of A per core
NK = N // P      # 32 contraction x-tiles
NM = MS // P     # 8 m-tiles per core
CW = 512         # output column chunk width (= 1 PSUM bank of fp32)
NCH = N // CW    # 8 chunks
R = 768          # truncated spectral rank (6 f-tiles)
NT = R // P      # 6 f-tiles

SIGMA = 8.0
TRUNC_EPS = 1e-10

_COMPILED = {}


def _band_profile():
    """g[d] = W2 band value at distance d (same fp32 path as the reference)."""
    d = np.arange(N).astype(np.float32)
    w1 = np.exp(-(d * d) / np.float32(2.0 * SIGMA * SIGMA)).astype(np.float32)
    w1 = np.where(w1 > np.float32(TRUNC_EPS), w1, np.float32(0.0))
    return (w1 * w1).astype(np.float64)


def _build_spectral():
    """Q [N, R] real-DFT basis, lam [R] eigenvalues, corner packs."""
    g = _band_profile()
    # circulant symbol c[k] = g(k) + g(N-k)
    c = g.copy()
    c[1:] += g[1:][::-1]
    lam = np.fft.rfft(c).real  # f = 0 .. N/2

    x = np.arange(N).astype(np.float64)
    fh = R // 2  # 384
    fc = np.arange(1, fh + 1)       # cos f = 1..384
    fs = np.arange(1, fh)           # sin f = 1..383
    q0 = np.full((N, 1), 1.0 / np.sqrt(N))
    qc = np.sqrt(2.0 / N) * np.cos(2 * np.pi * np.outer(x, fc) / N)
    qs = np.sqrt(2.0 / N) * np.sin(2 * np.pi * np.outer(x, fs) / N)
    Q = np.concatenate([q0, qc, qs], axis=1)               # [N, 768]
    lam_keep = np.concatenate([[lam[0]], lam[1:fh + 1], lam[1:fh]])

    # corner blocks of E = C - W2 (W2 is zero there):
    # Etr[i, j] = c[(i - j - (N - P)) mod N] = g(i + P - j) for i+P-j <= 54
    ii = np.arange(P)[:, None]
    jj = np.arange(P)[None, :]
    dtr = ii + P - jj
    etr = np.where((dtr >= 0) & (dtr <= 54), g[np.clip(dtr, 0, 54)], 0.0)
    ebl = etr.T
    return (
        Q.astype(np.float32),
        lam_keep.astype(np.float32),
        etr.astype(np.float32),
        ebl.astype(np.float32),
    )


def _build_program(reps=1):
    """Build + compile the Bass program (one NEFF, run SPMD on 8 cores)."""
    nc = bacc.Bacc("TRN2", target_bir_lowering=False, debug=False,
                   num_devices=NCORES)
    f32 = mybir.dt.float32
    bf16 = mybir.dt.bfloat16

    at_dram = nc.dram_tensor("at", [N, MS], bf16, kind="ExternalInput").ap()
    btsl_dram = nc.dram_tensor("btsl", [N, CW], bf16,
                               kind="ExternalInput").ap()
    qa_dram = nc.dram_tensor("qa", [N, R], bf16, kind="ExternalInput").ap()
    qb_dram = nc.dram_tensor("qb", [N, R], bf16, kind="ExternalInput").ap()
    ec_dram = nc.dram_tensor("ec", [P, 2 * P], bf16, kind="ExternalInput").ap()
    btc_dram = nc.dram_tensor("btc", [2 * P, N], bf16,
                              kind="ExternalInput").ap()
    out_dram = nc.dram_tensor("out", [MS, N], bf16, kind="ExternalOutput").ap()

    at3 = at_dram.rearrange("(x p) m -> p x m", p=P)      # [128, 32, 1024]
    btsl3 = btsl_dram.rearrange("(x p) c -> p x c", p=P)  # [128, 32, 512]
    qa3 = qa_dram.rearrange("(x p) f -> p x f", p=P)      # [128, 32, 768]
    qb3 = qb_dram.rearrange("(x p) f -> p x f", p=P)
    btc3 = btc_dram.rearrange("(t p) c -> p t c", p=P)    # [128, 2, 4096]
    out3 = out_dram.rearrange("(m p) c -> p m c", p=P)    # [128, 8, 4096]

    with TileContext(nc) as tc:
        with (
            tc.tile_pool(name="const", bufs=1) as const_pool,
            tc.tile_pool(name="resp", bufs=1) as res_pool,
            tc.tile_pool(name="qstr", bufs=2) as q_pool,
            tc.tile_pool(name="hxp", bufs=2) as hx_pool,
            tc.tile_pool(name="obw", bufs=2) as obw_pool,
            tc.tile_pool(name="psp", bufs=8, space="PSUM") as ps_pool,
        ):
            for rep in range(reps):
                sfx = f"_{rep}"
                # DRAM scratch for the gather (per-rep names)
                hs_dram = nc.dram_tensor(f"hs{sfx}", [R, CW], bf16,
                                         kind="Internal").ap()
                hg_dram = nc.dram_tensor(
                    f"hg{sfx}", [NCORES, R, CW], bf16, kind="Internal",
                    addr_space="Shared",
                ).ap()
                hs3 = hs_dram.rearrange("(t p) c -> p t c", p=P)

                # ---- resident loads
                ec_sb = const_pool.tile([P, 2 * P], bf16, tag="ec",
                                        name=f"ec_sb{sfx}")
                nc.sync.dma_start(ec_sb, ec_dram)
                btc_sb = const_pool.tile([P, 2 * N], bf16, tag="btc",
                                         name=f"btc_sb{sfx}")
                nc.sync.dma_start(
                    btc_sb.rearrange("p (t c) -> p t c", c=N), btc3)
                btsl_sb = res_pool.tile([P, NK * CW], bf16, tag="btsl",
                                        name=f"btsl_sb{sfx}")
                nc.sync.dma_start(
                    btsl_sb.rearrange("p (x c) -> p x c", c=CW), btsl3)
                at_sb = []
                for ha in range(2):
                    t = res_pool.tile([P, 16 * MS], bf16, tag=f"at{ha}",
                                      name=f"at_sb{sfx}_{ha}")
                    nc.sync.dma_start(
                        t.rearrange("p (x m) -> p x m", m=MS),
                        at3[:, ha * 16:(ha + 1) * 16, :])
                    at_sb.append(t)

                def at_t(x):
                    return at_sb[x // 16][:, (x % 16) * MS:(x % 16 + 1) * MS]

                # ---- stage H: HT_loc = qb.T @ btsl  -> hs_dram
                ps_h = [
                    ps_pool.tile([P, CW], f32, tag="ps", name=f"ps_h{sfx}_{t}")
                    for t in range(NT)
                ]
                QG = 4  # x-tiles per streamed q DMA
                for xg in range(NK // QG):
                    qb_t = q_pool.tile([P, QG * R], bf16, tag="qs",
                                       name=f"qb_sb{sfx}_{xg}")
                    nc.sync.dma_start(
                        qb_t.rearrange("p (x f) -> p x f", f=R),
                        qb3[:, xg * QG:(xg + 1) * QG, :])
                    for xi in range(QG):
                        x = xg * QG + xi
                        for t in range(NT):
                            nc.tensor.matmul(
                                ps_h[t],
                                lhsT=qb_t[:, xi * R + t * P:xi * R + (t + 1) * P],
                                rhs=btsl_sb[:, x * CW:(x + 1) * CW],
                                start=(x == 0),
                                stop=(x == NK - 1),
                            )
                hl_sb = hx_pool.tile([P, NT * CW], bf16, tag="hl",
                                     name=f"hl_sb{sfx}")
                for t in range(NT):
                    if t % 2 == 0:
                        nc.vector.tensor_copy(
                            hl_sb[:, t * CW:(t + 1) * CW], ps_h[t])
                    else:
                        nc.scalar.copy(
                            hl_sb[:, t * CW:(t + 1) * CW], ps_h[t])
                hs_wr = nc.sync.dma_start(
                    hs3, hl_sb.rearrange("p (t c) -> p t c", c=CW))
                cc = nc.gpsimd.collective_compute(
                    "AllGather",
                    mybir.AluOpType.bypass,
                    replica_groups=[list(range(NCORES))],
                    ins=[hs_dram],
                    outs=[hg_dram],
                )
                add_dep_helper(cc.ins, hs_wr.ins, reason="gather after hs write")

                # ---- stage G: GT = qa.T @ at (two f-groups to fit PSUM)
                gt_sb = [
                    res_pool.tile([P, MS], bf16, tag=f"gt{t}",
                                  name=f"gt_sb{sfx}_{t}")
                    for t in range(NT)
                ]
                for tg, tlist in enumerate((range(0, 4), range(4, NT))):
                    tlist = list(tlist)
                    ps_g = {
                        (t, mh): ps_pool.tile([P, CW], f32, tag="ps",
                                              name=f"ps_g{sfx}_{t}_{mh}")
                        for t in tlist for mh in range(2)
                    }
                    f0 = tlist[0] * P
                    fw = len(tlist) * P
                    for xg in range(NK // QG):
                        qa_t = q_pool.tile([P, QG * fw], bf16, tag="qs",
                                           name=f"qa_sb{sfx}_{tg}_{xg}")
                        nc.sync.dma_start(
                            qa_t.rearrange("p (x f) -> p x f", f=fw),
                            qa3[:, xg * QG:(xg + 1) * QG, f0:f0 + fw])
                        for xi in range(QG):
                            x = xg * QG + xi
                            for ti, t in enumerate(tlist):
                                for mh in range(2):
                                    nc.tensor.matmul(
                                        ps_g[(t, mh)],
                                        lhsT=qa_t[:, xi * fw + ti * P:
                                                  xi * fw + (ti + 1) * P],
                                        rhs=at_t(x)[:, mh * CW:(mh + 1) * CW],
                                        start=(x == 0),
                                        stop=(x == NK - 1),
                                    )
                    for ti, t in enumerate(tlist):
                        for mh in range(2):
                            hs_ = bass.ts(mh, CW)
                            if (ti + mh) % 2 == 0:
                                nc.vector.tensor_copy(
                                    gt_sb[t][:, hs_], ps_g[(t, mh)])
                            else:
                                nc.scalar.copy(
                                    gt_sb[t][:, hs_], ps_g[(t, mh)])

                # ---- corner factors: GTc = (-E_corner).T @ at_corner
                gtc_sb = []
                for ci, x in enumerate((0, NK - 1)):
                    g_sb = res_pool.tile([P, MS], bf16, tag=f"gtc{ci}",
                                         name=f"gtc_sb{sfx}_{ci}")
                    for mh in range(2):
                        ps_c = ps_pool.tile([P, CW], f32, tag="ps",
                                            name=f"ps_c{sfx}_{ci}_{mh}")
                        nc.tensor.matmul(
                            ps_c,
                            lhsT=ec_sb[:, ci * P:(ci + 1) * P],
                            rhs=at_t(x)[:, mh * CW:(mh + 1) * CW],
                            start=True,
                            stop=True,
                        )
                        nc.vector.tensor_copy(
                            g_sb[:, mh * CW:(mh + 1) * CW], ps_c)
                    gtc_sb.append(g_sb)

                # ---- final: out = GT.T @ HT + GTc.T @ btc, per 512-col chunk
                for nu in range(NCH):
                    cs = bass.ts(nu, CW)
                    hn_t = hx_pool.tile([P, NT * CW], bf16, tag="hn",
                                        name=f"hn_sb{sfx}_{nu}")
                    hn_rd = nc.sync.dma_start(
                        hn_t.rearrange("p (t c) -> p t c", c=CW),
                        hg_dram[nu].rearrange("(t p) c -> p t c", p=P),
                    )
                    add_dep_helper(hn_rd.ins, cc.ins, reason="read gathered HT")
                    ps_o = [
                        ps_pool.tile([P, CW], f32, tag="ps",
                                     name=f"ps_o{sfx}_{nu}_{m}")
                        for m in range(NM)
                    ]
                    for t in range(NT):
                        for m in range(NM):
                            nc.tensor.matmul(
                                ps_o[m],
                                lhsT=gt_sb[t][:, m * P:(m + 1) * P],
                                rhs=hn_t[:, t * CW:(t + 1) * CW],
                                start=(t == 0),
                                stop=False,
                            )
                    for ci in range(2):
                        for m in range(NM):
                            nc.tensor.matmul(
                                ps_o[m],
                                lhsT=gtc_sb[ci][:, m * P:(m + 1) * P],
                                rhs=btc_sb[:, ci * N + nu * CW:
                                           ci * N + (nu + 1) * CW],
                                start=False,
                                stop=(ci == 1),
                            )
                    obw_t = obw_pool.tile([P, NM * CW], bf16, tag="obw",
                                          name=f"obw_sb{sfx}_{nu}")
                    for m in range(NM):
                        if m % 2 == 0:
                            nc.vector.tensor_copy(
                                obw_t[:, m * CW:(m + 1) * CW], ps_o[m])
                        else:
                            nc.scalar.copy(
                                obw_t[:, m * CW:(m + 1) * CW], ps_o[m])
                    nc.sync.dma_start(
                        out3[:, :, cs],
                        obw_t.rearrange("p (m c) -> p m c", c=CW),
                    )

    nc.compile()
    return nc


def _get_program():
    if "nc" not in _COMPILED:
        _COMPILED["nc"] = _build_program()
    return _COMPILED["nc"]


def _build_in_maps(A, B):
    import ml_dtypes

    A = np.asarray(A, dtype=np.float32)
    B = np.asarray(B, dtype=np.float32)
    assert A.shape == (M_FULL, N), A.shape
    assert B.shape == (N, N), B.shape

    Q, lam, etr, ebl = _build_spectral()
    bf = ml_dtypes.bfloat16

    a_t = np.ascontiguousarray(A.T.astype(bf))              # [4096, 8192]
    b_t = np.ascontiguousarray(B.T.astype(bf))              # [4096, 4096]
    qa = np.ascontiguousarray(Q.astype(bf))                 # [4096, 768]
    qb = np.ascontiguousarray((Q * lam[None, :]).astype(bf))
    ec = np.ascontiguousarray(
        np.concatenate([-etr, -ebl], axis=1).astype(bf))    # [128, 256]
    btc = np.ascontiguousarray(
        np.concatenate([b_t[N - P:N, :], b_t[0:P, :]], axis=0).astype(bf))

    return [
        {
            "at": np.ascontiguousarray(a_t[:, c * MS:(c + 1) * MS]),
            "btsl": np.ascontiguousarray(b_t[:, c * CW:(c + 1) * CW]),
            "qa": qa,
            "qb": qb,
            "ec": ec,
            "btc": btc,
        }
        for c in range(NCORES)
    ]


def kernel(A, B):
    in_maps = _build_in_maps(A, B)
    nc = _get_program()
    res = run_bass_kernel_spmd(nc, in_maps, core_ids=list(range(NCORES)))
    return np.concatenate(
        [np.asarray(res.results[c]["out"]) for c in range(NCORES)], axis=0
    ).astype(np.float32)


# revision 14
# speedup vs baseline: 4.0448x; 2.3406x over previous
"""Trainium2 kernel for out = A @ W2 @ B.T with banded Gaussian W2.

Math: W2 = W1*W1, W1[i,j] = exp(-(i-j)^2/(2*8^2)) truncated below 1e-10, so
W2 = exp(-(i-j)^2/64) on a |i-j| <= 54 band. W2 is symmetric banded Toeplitz.

Embed W2 in the 4096-circulant C with the same band symbol:
    C = W2 + E,  E = two 54x54 corner triangles (the wrap-around band).
C diagonalizes in the real DFT basis: C = Q_full diag(lam) Q_full.T with
lam(f) ~ sqrt(64*pi) * exp(-16 * (2*pi*f/N)^2): only the lowest ~384
frequencies matter. Keeping r = 768 basis vectors (f=0, cos/sin f=1..383,
cos f=384) gives truncation error ~1.5e-3 (validated numerically), so

    out = (A @ Q) @ (B @ Q*lam).T - A @ E @ B.T

cuts the big-GEMM FLOPs ~2.6x vs A @ W2 @ B.T done densely over 4096.

Distribution (8 cores, one AllGather):
  - core c holds: A.T slab [4096, 1024] (its 1024 A-rows), B.T column slice
    [4096, 512], Q and Q*lam [4096, 768], corner packs (all bf16).
  - stage H (sharded): HT_loc = (Q*lam).T @ B.T[:, c-slice]  [768, 512]
    -> DRAM AllGather -> HT full [768, 4096] available to every core.
  - stage G (data-parallel): GT = Q.T @ A.T_slab  [768, 1024], plus the two
    corner factors GTc = (-E_corner).T @ A.T_corner [128, 1024] each.
  - final: out_slab = GT.T @ HT + GTc.T @ B.T_corner, streamed per 512-col
    chunk with all 8 PSUM banks accumulating the 8 m-tiles.
  - all matmuls bf16 (1 cyc/row, FP32 PSUM accumulate); out stored bf16 and
    upcast on host (adds ~2e-3 rounding, total rel err ~5e-3 < 2e-2).
"""

import numpy as np

import concourse.bass as bass
import concourse.mybir as mybir
from concourse import bacc
from concourse.bass_utils import run_bass_kernel_spmd
from concourse.tile import TileContext
from concourse.tile_rust import add_dep_helper

P = 128          # partition / block size
N = 4096         # inner dims (A cols, B rows/cols)
M_FULL = 8192    # A rows
NCORES = 8
MS = M_FULL // NCORES   # 1024 rows of A per core
NK = N // P      # 32 contraction x-tiles
NM = MS // P     # 8 m-tiles per core
CW = 512         # output column chunk width (= 1 PSUM bank of fp32)
NCH = N // CW    # 8 chunks
R = 768          # truncated spectral rank (6 f-tiles)
NT = R // P      # 6 f-tiles

SIGMA = 8.0
TRUNC_EPS = 1e-10

_COMPILED = {}


def _band_profile():
    """g[d] = W2 band value at distance d (same fp32 path as the reference)."""
    d = np.arange(N).astype(np.float32)
    w1 = np.exp(-(d * d) / np.float32(2.0 * SIGMA * SIGMA)).astype(np.float32)
    w1 = np.where(w1 > np.float32(TRUNC_EPS), w1, np.float32(0.0))
    return (w1 * w1).astype(np.float64)


def _build_spectral():
    """Q [N, R] real-DFT basis, lam [R] eigenvalues, corner packs."""
    g = _band_profile()
    # circulant symbol c[k] = g(k) + g(N-k)
    c = g.copy()
    c[1:] += g[1:][::-1]
    lam = np.fft.rfft(c).real  # f = 0 .. N/2

    x = np.arange(N).astype(np.float64)
    fh = R // 2  # 384
    fc = np.arange(1, fh + 1)       # cos f = 1..384
    fs = np.arange(1, fh)           # sin f = 1..383
    q0 = np.full((N, 1), 1.0 / np.sqrt(N))
    qc = np.sqrt(2.0 / N) * np.cos(2 * np.pi * np.outer(x, fc) / N)
    qs = np.sqrt(2.0 / N) * np.sin(2 * np.pi * np.outer(x, fs) / N)
    Q = np.concatenate([q0, qc, qs], axis=1)               # [N, 768]
    lam_keep = np.concatenate([[lam[0]], lam[1:fh + 1], lam[1:fh]])

    # corner blocks of E = C - W2 (W2 is zero there):
    # Etr[i, j] = c[(i - j - (N - P)) mod N] = g(i + P - j) for i+P-j <= 54
    ii = np.arange(P)[:, None]
    jj = np.arange(P)[None, :]
    dtr = ii + P - jj
    etr = np.where((dtr >= 0) & (dtr <= 54), g[np.clip(dtr, 0, 54)], 0.0)
    ebl = etr.T
    return (
        Q.astype(np.float32),
        lam_keep.astype(np.float32),
        etr.astype(np.float32),
        ebl.astype(np.float32),
    )


def _build_program(reps=1):
    """Build + compile the Bass program (one NEFF, run SPMD on 8 cores)."""
    nc = bacc.Bacc("TRN2", target_bir_lowering=False, debug=False,
                   num_devices=NCORES)
    f32 = mybir.dt.float32
    bf16 = mybir.dt.bfloat16

    at_dram = nc.dram_tensor("at", [N, MS], bf16, kind="ExternalInput").ap()
    btsl_dram = nc.dram_tensor("btsl", [N, CW], bf16,
                               kind="ExternalInput").ap()
    qa_dram = nc.dram_tensor("qa", [N, R], bf16, kind="ExternalInput").ap()
    qb_dram = nc.dram_tensor("qb", [N, R], bf16, kind="ExternalInput").ap()
    ec_dram = nc.dram_tensor("ec", [P, 2 * P], bf16, kind="ExternalInput").ap()
    btc_dram = nc.dram_tensor("btc", [P, N], bf16,
                              kind="ExternalInput").ap()
    out_dram = nc.dram_tensor("out", [MS, N], bf16, kind="ExternalOutput").ap()

    at3 = at_dram.rearrange("(x p) m -> p x m", p=P)      # [128, 32, 1024]
    btsl3 = btsl_dram.rearrange("(x p) c -> p x c", p=P)  # [128, 32, 512]
    qa3 = qa_dram.rearrange("(x p) f -> p x f", p=P)      # [128, 32, 768]
    qb3 = qb_dram.rearrange("(x p) f -> p x f", p=P)
    out3 = out_dram.rearrange("(m p) c -> p m c", p=P)    # [128, 8, 4096]

    with TileContext(nc) as tc:
        with (
            tc.tile_pool(name="const", bufs=1) as const_pool,
            tc.tile_pool(name="resp", bufs=1) as res_pool,
            tc.tile_pool(name="qstr", bufs=2) as q_pool,
            tc.tile_pool(name="hxp", bufs=2) as hx_pool,
            tc.tile_pool(name="obw", bufs=2) as obw_pool,
            tc.tile_pool(name="psp", bufs=8, space="PSUM") as ps_pool,
        ):
            for rep in range(reps):
                sfx = f"_{rep}"
                # DRAM scratch for the gather (per-rep names)
                hs_dram = nc.dram_tensor(f"hs{sfx}", [R, CW], bf16,
                                         kind="Internal").ap()
                hg_dram = nc.dram_tensor(
                    f"hg{sfx}", [NCORES, R, CW], bf16, kind="Internal",
                    addr_space="Shared",
                ).ap()
                hs3 = hs_dram.rearrange("(t p) c -> p t c", p=P)

                # ---- resident loads
                ec_sb = const_pool.tile([P, 2 * P], bf16, tag="ec",
                                        name=f"ec_sb{sfx}")
                nc.sync.dma_start(ec_sb, ec_dram)
                btc_sb = const_pool.tile([P, N], bf16, tag="btc",
                                         name=f"btc_sb{sfx}")
                nc.sync.dma_start(btc_sb, btc_dram)
                btsl_sb = res_pool.tile([P, NK * CW], bf16, tag="btsl",
                                        name=f"btsl_sb{sfx}")
                btsl_v = btsl_sb.rearrange("p (x c) -> p x c", c=CW)
                for bg in range(4):
                    nc.sync.dma_start(
                        btsl_v[:, bg * 8:(bg + 1) * 8, :],
                        btsl3[:, bg * 8:(bg + 1) * 8, :])
                at_sb = []
                for ha in range(2):
                    t = res_pool.tile([P, 16 * MS], bf16, tag=f"at{ha}",
                                      name=f"at_sb{sfx}_{ha}")
                    nc.sync.dma_start(
                        t.rearrange("p (x m) -> p x m", m=MS),
                        at3[:, ha * 16:(ha + 1) * 16, :])
                    at_sb.append(t)

                def at_t(x):
                    return at_sb[x // 16][:, (x % 16) * MS:(x % 16 + 1) * MS]

                # ---- stage H: HT_loc = qb.T @ btsl  -> hs_dram
                ps_h = [
                    ps_pool.tile([P, CW], f32, tag="ps", name=f"ps_h{sfx}_{t}")
                    for t in range(NT)
                ]
                QG = 4  # x-tiles per streamed q DMA
                for xg in range(NK // QG):
                    qb_t = q_pool.tile([P, QG * R], bf16, tag="qs",
                                       name=f"qb_sb{sfx}_{xg}")
                    nc.sync.dma_start(
                        qb_t.rearrange("p (x f) -> p x f", f=R),
                        qb3[:, xg * QG:(xg + 1) * QG, :])
                    for xi in range(QG):
                        x = xg * QG + xi
                        for t in range(NT):
                            nc.tensor.matmul(
                                ps_h[t],
                                lhsT=qb_t[:, xi * R + t * P:xi * R + (t + 1) * P],
                                rhs=btsl_sb[:, x * CW:(x + 1) * CW],
                                start=(x == 0),
                                stop=(x == NK - 1),
                            )
                hl_sb = hx_pool.tile([P, NT * CW], bf16, tag="hl",
                                     name=f"hl_sb{sfx}")
                for t in range(NT):
                    if t % 2 == 0:
                        nc.vector.tensor_copy(
                            hl_sb[:, t * CW:(t + 1) * CW], ps_h[t])
                    else:
                        nc.scalar.copy(
                            hl_sb[:, t * CW:(t + 1) * CW], ps_h[t])
                hs_wr = nc.sync.dma_start(
                    hs3, hl_sb.rearrange("p (t c) -> p t c", c=CW))
                cc = nc.gpsimd.collective_compute(
                    "AllGather",
                    mybir.AluOpType.bypass,
                    replica_groups=[list(range(NCORES))],
                    ins=[hs_dram],
                    outs=[hg_dram],
                )
                add_dep_helper(cc.ins, hs_wr.ins, reason="gather after hs write")

                # ---- stage G: GT = qa.T @ at (two f-groups to fit PSUM)
                gt_sb = [
                    res_pool.tile([P, MS], bf16, tag=f"gt{t}",
                                  name=f"gt_sb{sfx}_{t}")
                    for t in range(NT)
                ]
                for tg, tlist in enumerate((range(0, 4), range(4, NT))):
                    tlist = list(tlist)
                    ps_g = {
                        (t, mh): ps_pool.tile([P, CW], f32, tag="ps",
                                              name=f"ps_g{sfx}_{t}_{mh}")
                        for t in tlist for mh in range(2)
                    }
                    f0 = tlist[0] * P
                    fw = len(tlist) * P
                    for xg in range(NK // QG):
                        qa_t = q_pool.tile([P, QG * fw], bf16, tag="qs",
                                           name=f"qa_sb{sfx}_{tg}_{xg}")
                        nc.sync.dma_start(
                            qa_t.rearrange("p (x f) -> p x f", f=fw),
                            qa3[:, xg * QG:(xg + 1) * QG, f0:f0 + fw])
                        for xi in range(QG):
                            x = xg * QG + xi
                            for ti, t in enumerate(tlist):
                                for mh in range(2):
                                    nc.tensor.matmul(
                                        ps_g[(t, mh)],
                                        lhsT=qa_t[:, xi * fw + ti * P:
                                                  xi * fw + (ti + 1) * P],
                                        rhs=at_t(x)[:, mh * CW:(mh + 1) * CW],
                                        start=(x == 0),
                                        stop=(x == NK - 1),
                                    )
                    for ti, t in enumerate(tlist):
                        for mh in range(2):
                            hs_ = bass.ts(mh, CW)
                            if (ti + mh) % 2 == 0:
                                nc.vector.tensor_copy(
                                    gt_sb[t][:, hs_], ps_g[(t, mh)])
                            else:
                                nc.scalar.copy(
                                    gt_sb[t][:, hs_], ps_g[(t, mh)])

                # ---- corner factor (both corners have disjoint row support,
                # packed into ONE [128, MS] tile):
                #   GTc = (-Etr).T @ at[0] + (-Ebl).T @ at[31]
                gtc_sb = res_pool.tile([P, MS], bf16, tag="gtc",
                                       name=f"gtc_sb{sfx}")
                for mh in range(2):
                    ps_c = ps_pool.tile([P, CW], f32, tag="ps",
                                        name=f"ps_c{sfx}_{mh}")
                    for ci, x in enumerate((0, NK - 1)):
                        nc.tensor.matmul(
                            ps_c,
                            lhsT=ec_sb[:, ci * P:(ci + 1) * P],
                            rhs=at_t(x)[:, mh * CW:(mh + 1) * CW],
                            start=(ci == 0),
                            stop=(ci == 1),
                        )
                    nc.vector.tensor_copy(
                        gtc_sb[:, mh * CW:(mh + 1) * CW], ps_c)

                # ---- final: out = GT.T @ HT + GTc.T @ btc, per 512-col chunk
                for nu in range(NCH):
                    cs = bass.ts(nu, CW)
                    hn_t = hx_pool.tile([P, NT * CW], bf16, tag="hn",
                                        name=f"hn_sb{sfx}_{nu}")
                    hn_rd = nc.sync.dma_start(
                        hn_t.rearrange("p (t c) -> p t c", c=CW),
                        hg_dram[nu].rearrange("(t p) c -> p t c", p=P),
                    )
                    add_dep_helper(hn_rd.ins, cc.ins, reason="read gathered HT")
                    ps_o = [
                        ps_pool.tile([P, CW], f32, tag="ps",
                                     name=f"ps_o{sfx}_{nu}_{m}")
                        for m in range(NM)
                    ]
                    # corner contribution first: it only needs resident
                    # tiles, so it can run before the gather lands
                    for m in range(NM):
                        nc.tensor.matmul(
                            ps_o[m],
                            lhsT=gtc_sb[:, m * P:(m + 1) * P],
                            rhs=btc_sb[:, cs],
                            start=True,
                            stop=False,
                        )
                    for t in range(NT):
                        for m in range(NM):
                            nc.tensor.matmul(
                                ps_o[m],
                                lhsT=gt_sb[t][:, m * P:(m + 1) * P],
                                rhs=hn_t[:, t * CW:(t + 1) * CW],
                                start=False,
                                stop=(t == NT - 1),
                            )
                    obw_t = obw_pool.tile([P, NM * CW], bf16, tag="obw",
                                          name=f"obw_sb{sfx}_{nu}")
                    for m in range(NM):
                        if m % 2 == 0:
                            nc.vector.tensor_copy(
                                obw_t[:, m * CW:(m + 1) * CW], ps_o[m])
                        else:
                            nc.scalar.copy(
                                obw_t[:, m * CW:(m + 1) * CW], ps_o[m])
                    nc.sync.dma_start(
                        out3[:, :, cs],
                        obw_t.rearrange("p (m c) -> p m c", c=CW),
                    )

    nc.compile()
    return nc


def _get_program():
    if "nc" not in _COMPILED:
        _COMPILED["nc"] = _build_program()
    return _COMPILED["nc"]


def _build_in_maps(A, B):
    import ml_dtypes

    A = np.asarray(A, dtype=np.float32)
    B = np.asarray(B, dtype=np.float32)
    assert A.shape == (M_FULL, N), A.shape
    assert B.shape == (N, N), B.shape

    Q, lam, etr, ebl = _build_spectral()
    bf = ml_dtypes.bfloat16

    a_t = np.ascontiguousarray(A.T.astype(bf))              # [4096, 8192]
    b_t = np.ascontiguousarray(B.T.astype(bf))              # [4096, 4096]
    qa = np.ascontiguousarray(Q.astype(bf))                 # [4096, 768]
    qb = np.ascontiguousarray((Q * lam[None, :]).astype(bf))
    ec = np.ascontiguousarray(
        np.concatenate([-etr, -ebl], axis=1).astype(bf))    # [128, 256]
    # packed corner B.T rows: j<64 -> B.T[j] (for Ebl, support j<=53);
    # j>=64 -> B.T[3968+j] (for Etr, support j>=74)
    btc = np.ascontiguousarray(
        np.concatenate([b_t[0:64, :], b_t[N - 64:N, :]], axis=0).astype(bf))

    return [
        {
            "at": np.ascontiguousarray(a_t[:, c * MS:(c + 1) * MS]),
            "btsl": np.ascontiguousarray(b_t[:, c * CW:(c + 1) * CW]),
            "qa": qa,
            "qb": qb,
            "ec": ec,
            "btc": btc,
        }
        for c in range(NCORES)
    ]


def kernel(A, B):
    in_maps = _build_in_maps(A, B)
    nc = _get_program()
    res = run_bass_kernel_spmd(nc, in_maps, core_ids=list(range(NCORES)))
    return np.concatenate(
        [np.asarray(res.results[c]["out"]) for c in range(NCORES)], axis=0
    ).astype(np.float32)


# revision 15
# speedup vs baseline: 9.2679x; 2.2913x over previous
"""Trainium2 kernel for out = A @ W2 @ B.T with banded Gaussian W2.

Math: W2 = W1*W1, W1[i,j] = exp(-(i-j)^2/(2*8^2)) truncated below 1e-10, so
W2 = exp(-(i-j)^2/64) on a |i-j| <= 54 band. W2 is symmetric banded Toeplitz.

Embed W2 in the 4096-circulant C with the same band symbol:
    C = W2 + E,  E = two 54x54 corner triangles (the wrap-around band).
C diagonalizes in the real DFT basis: C = Q_full diag(lam) Q_full.T with
lam(f) ~ sqrt(64*pi) * exp(-16 * (2*pi*f/N)^2): only the lowest ~384
frequencies matter. Keeping r = 768 basis vectors (f=0, cos/sin f=1..383,
cos f=384) gives truncation error ~1.5e-3 (validated numerically), so

    out = (A @ Q) @ (B @ Q*lam).T - A @ E @ B.T

cuts the big-GEMM FLOPs ~2.6x vs A @ W2 @ B.T done densely over 4096.

Distribution (8 cores, one AllGather):
  - core c holds: A.T slab [4096, 1024] (its 1024 A-rows), B.T column slice
    [4096, 512], Q and Q*lam [4096, 768], corner packs (all bf16).
  - stage H (sharded): HT_loc = (Q*lam).T @ B.T[:, c-slice]  [768, 512]
    -> DRAM AllGather -> HT full [768, 4096] available to every core.
  - stage G (data-parallel): GT = Q.T @ A.T_slab  [768, 1024], plus the two
    corner factors GTc = (-E_corner).T @ A.T_corner [128, 1024] each.
  - final: out_slab = GT.T @ HT + GTc.T @ B.T_corner, streamed per 512-col
    chunk with all 8 PSUM banks accumulating the 8 m-tiles.
  - all matmuls bf16 (1 cyc/row, FP32 PSUM accumulate); out stored bf16 and
    upcast on host (adds ~2e-3 rounding, total rel err ~5e-3 < 2e-2).
"""

import numpy as np

import concourse.bass as bass
import concourse.mybir as mybir
from concourse import bacc
from concourse.bass_utils import run_bass_kernel_spmd
from concourse.tile import TileContext
from concourse.tile_rust import add_dep_helper

P = 128          # partition / block size
N = 4096         # inner dims (A cols, B rows/cols)
M_FULL = 8192    # A rows
NCORES = 8
MS = M_FULL // NCORES   # 1024 rows of A per core
NK = N // P      # 32 contraction x-tiles
NM = MS // P     # 8 m-tiles per core
CW = 512         # output column chunk width (= 1 PSUM bank of fp32)
NCH = N // CW    # 8 chunks
R = 768          # truncated spectral rank (6 f-tiles)
NT = R // P      # 6 f-tiles

SIGMA = 8.0
TRUNC_EPS = 1e-10

_COMPILED = {}


def _band_profile():
    """g[d] = W2 band value at distance d (same fp32 path as the reference)."""
    d = np.arange(N).astype(np.float32)
    w1 = np.exp(-(d * d) / np.float32(2.0 * SIGMA * SIGMA)).astype(np.float32)
    w1 = np.where(w1 > np.float32(TRUNC_EPS), w1, np.float32(0.0))
    return (w1 * w1).astype(np.float64)


def _build_spectral():
    """Q [N, R] real-DFT basis, lam [R] eigenvalues, corner packs."""
    g = _band_profile()
    # circulant symbol c[k] = g(k) + g(N-k)
    c = g.copy()
    c[1:] += g[1:][::-1]
    lam = np.fft.rfft(c).real  # f = 0 .. N/2

    x = np.arange(N).astype(np.float64)
    fh = R // 2  # 384
    fc = np.arange(1, fh + 1)       # cos f = 1..384
    fs = np.arange(1, fh)           # sin f = 1..383
    q0 = np.full((N, 1), 1.0 / np.sqrt(N))
    qc = np.sqrt(2.0 / N) * np.cos(2 * np.pi * np.outer(x, fc) / N)
    qs = np.sqrt(2.0 / N) * np.sin(2 * np.pi * np.outer(x, fs) / N)
    Q = np.concatenate([q0, qc, qs], axis=1)               # [N, 768]
    lam_keep = np.concatenate([[lam[0]], lam[1:fh + 1], lam[1:fh]])

    # corner blocks of E = C - W2 (W2 is zero there):
    # Etr[i, j] = c[(i - j - (N - P)) mod N] = g(i + P - j) for i+P-j <= 54
    ii = np.arange(P)[:, None]
    jj = np.arange(P)[None, :]
    dtr = ii + P - jj
    etr = np.where((dtr >= 0) & (dtr <= 54), g[np.clip(dtr, 0, 54)], 0.0)
    ebl = etr.T
    return (
        Q.astype(np.float32),
        lam_keep.astype(np.float32),
        etr.astype(np.float32),
        ebl.astype(np.float32),
    )


def _build_program(reps=1):
    """Build + compile the Bass program (one NEFF, run SPMD on 8 cores)."""
    nc = bacc.Bacc("TRN2", target_bir_lowering=False, debug=False,
                   num_devices=NCORES)
    f32 = mybir.dt.float32
    bf16 = mybir.dt.bfloat16

    at_dram = nc.dram_tensor("at", [N, MS], bf16, kind="ExternalInput").ap()
    btsl_dram = nc.dram_tensor("btsl", [N, CW], bf16,
                               kind="ExternalInput").ap()
    qa_dram = nc.dram_tensor("qa", [N, R], bf16, kind="ExternalInput").ap()
    qb_dram = nc.dram_tensor("qb", [N, R], bf16, kind="ExternalInput").ap()
    ec_dram = nc.dram_tensor("ec", [P, 2 * P], bf16, kind="ExternalInput").ap()
    btc_dram = nc.dram_tensor("btc", [P, N], bf16,
                              kind="ExternalInput").ap()
    out_dram = nc.dram_tensor("out", [MS, N], bf16, kind="ExternalOutput").ap()

    at3 = at_dram.rearrange("(x p) m -> p x m", p=P)      # [128, 32, 1024]
    btsl3 = btsl_dram.rearrange("(x p) c -> p x c", p=P)  # [128, 32, 512]
    qa3 = qa_dram.rearrange("(x p) f -> p x f", p=P)      # [128, 32, 768]
    qb3 = qb_dram.rearrange("(x p) f -> p x f", p=P)
    out3 = out_dram.rearrange("(m p) c -> p m c", p=P)    # [128, 8, 4096]

    with TileContext(nc) as tc:
        with (
            tc.tile_pool(name="const", bufs=1) as const_pool,
            tc.tile_pool(name="resp", bufs=1) as res_pool,
            tc.tile_pool(name="qstr", bufs=2) as q_pool,
            tc.tile_pool(name="hxp", bufs=2) as hx_pool,
            tc.tile_pool(name="obw", bufs=2) as obw_pool,
            tc.tile_pool(name="psp", bufs=8, space="PSUM") as ps_pool,
        ):
            for rep in range(reps):
                sfx = f"_{rep}"
                # DRAM scratch for the gather (per-rep names)
                hs_dram = nc.dram_tensor(f"hs{sfx}", [R, CW], bf16,
                                         kind="Internal").ap()
                hg_dram = nc.dram_tensor(
                    f"hg{sfx}", [NCORES, R, CW], bf16, kind="Internal",
                    addr_space="Shared",
                ).ap()
                hs3 = hs_dram.rearrange("(t p) c -> p t c", p=P)

                # ---- resident loads, issued in first-use order:
                # btsl feeds stage H immediately; at/ec/btc are needed only
                # from stage G onwards and are issued after the H loop so
                # they don't queue ahead of H's data.
                btsl_sb = res_pool.tile([P, NK * CW], bf16, tag="btsl",
                                        name=f"btsl_sb{sfx}")
                btsl_v = btsl_sb.rearrange("p (x c) -> p x c", c=CW)
                for bg in range(4):
                    nc.sync.dma_start(
                        btsl_v[:, bg * 8:(bg + 1) * 8, :],
                        btsl3[:, bg * 8:(bg + 1) * 8, :])

                # ---- stage H: HT_loc = qb.T @ btsl  -> hs_dram
                ps_h = [
                    ps_pool.tile([P, CW], f32, tag="ps", name=f"ps_h{sfx}_{t}")
                    for t in range(NT)
                ]
                QG = 4  # x-tiles per streamed q DMA
                for xg in range(NK // QG):
                    qb_t = q_pool.tile([P, QG * R], bf16, tag="qs",
                                       name=f"qb_sb{sfx}_{xg}")
                    nc.sync.dma_start(
                        qb_t.rearrange("p (x f) -> p x f", f=R),
                        qb3[:, xg * QG:(xg + 1) * QG, :])
                    for xi in range(QG):
                        x = xg * QG + xi
                        for t in range(NT):
                            nc.tensor.matmul(
                                ps_h[t],
                                lhsT=qb_t[:, xi * R + t * P:xi * R + (t + 1) * P],
                                rhs=btsl_sb[:, x * CW:(x + 1) * CW],
                                start=(x == 0),
                                stop=(x == NK - 1),
                            )
                hl_sb = hx_pool.tile([P, NT * CW], bf16, tag="hl",
                                     name=f"hl_sb{sfx}")
                for t in range(NT):
                    nc.vector.tensor_copy(
                        hl_sb[:, t * CW:(t + 1) * CW], ps_h[t])
                hs_wr = nc.sync.dma_start(
                    hs3, hl_sb.rearrange("p (t c) -> p t c", c=CW))

                # deferred resident loads (consumed from stage G onwards)
                at_sb = []
                for ha in range(2):
                    t = res_pool.tile([P, 16 * MS], bf16, tag=f"at{ha}",
                                      name=f"at_sb{sfx}_{ha}")
                    nc.sync.dma_start(
                        t.rearrange("p (x m) -> p x m", m=MS),
                        at3[:, ha * 16:(ha + 1) * 16, :])
                    at_sb.append(t)

                def at_t(x):
                    return at_sb[x // 16][:, (x % 16) * MS:(x % 16 + 1) * MS]

                ec_sb = const_pool.tile([P, 2 * P], bf16, tag="ec",
                                        name=f"ec_sb{sfx}")
                nc.sync.dma_start(ec_sb, ec_dram)
                btc_sb = const_pool.tile([P, N], bf16, tag="btc",
                                         name=f"btc_sb{sfx}")
                nc.sync.dma_start(btc_sb, btc_dram)
                cc = nc.gpsimd.collective_compute(
                    "AllGather",
                    mybir.AluOpType.bypass,
                    replica_groups=[list(range(NCORES))],
                    ins=[hs_dram],
                    outs=[hg_dram],
                )
                add_dep_helper(cc.ins, hs_wr.ins, reason="gather after hs write")

                # ---- stage G: GT = qa.T @ at (two f-groups to fit PSUM)
                gt_sb = [
                    res_pool.tile([P, MS], bf16, tag=f"gt{t}",
                                  name=f"gt_sb{sfx}_{t}")
                    for t in range(NT)
                ]
                for tg, tlist in enumerate((range(0, 4), range(4, NT))):
                    tlist = list(tlist)
                    ps_g = {
                        (t, mh): ps_pool.tile([P, CW], f32, tag="ps",
                                              name=f"ps_g{sfx}_{t}_{mh}")
                        for t in tlist for mh in range(2)
                    }
                    f0 = tlist[0] * P
                    fw = len(tlist) * P
                    for xg in range(NK // QG):
                        qa_t = q_pool.tile([P, QG * fw], bf16, tag="qs",
                                           name=f"qa_sb{sfx}_{tg}_{xg}")
                        nc.sync.dma_start(
                            qa_t.rearrange("p (x f) -> p x f", f=fw),
                            qa3[:, xg * QG:(xg + 1) * QG, f0:f0 + fw])
                        for xi in range(QG):
                            x = xg * QG + xi
                            for ti, t in enumerate(tlist):
                                for mh in range(2):
                                    nc.tensor.matmul(
                                        ps_g[(t, mh)],
                                        lhsT=qa_t[:, xi * fw + ti * P:
                                                  xi * fw + (ti + 1) * P],
                                        rhs=at_t(x)[:, mh * CW:(mh + 1) * CW],
                                        start=(x == 0),
                                        stop=(x == NK - 1),
                                    )
                    for ti, t in enumerate(tlist):
                        for mh in range(2):
                            hs_ = bass.ts(mh, CW)
                            nc.vector.tensor_copy(
                                gt_sb[t][:, hs_], ps_g[(t, mh)])

                # ---- corner factor (both corners have disjoint row support,
                # packed into ONE [128, MS] tile):
                #   GTc = (-Etr).T @ at[0] + (-Ebl).T @ at[31]
                gtc_sb = res_pool.tile([P, MS], bf16, tag="gtc",
                                       name=f"gtc_sb{sfx}")
                for mh in range(2):
                    ps_c = ps_pool.tile([P, CW], f32, tag="ps",
                                        name=f"ps_c{sfx}_{mh}")
                    for ci, x in enumerate((0, NK - 1)):
                        nc.tensor.matmul(
                            ps_c,
                            lhsT=ec_sb[:, ci * P:(ci + 1) * P],
                            rhs=at_t(x)[:, mh * CW:(mh + 1) * CW],
                            start=(ci == 0),
                            stop=(ci == 1),
                        )
                    nc.vector.tensor_copy(
                        gtc_sb[:, mh * CW:(mh + 1) * CW], ps_c)

                # ---- final: out = GT.T @ HT + GTc.T @ btc, per 512-col chunk
                for nu in range(NCH):
                    cs = bass.ts(nu, CW)
                    hn_t = hx_pool.tile([P, NT * CW], bf16, tag="hn",
                                        name=f"hn_sb{sfx}_{nu}")
                    hn_rd = nc.sync.dma_start(
                        hn_t.rearrange("p (t c) -> p t c", c=CW),
                        hg_dram[nu].rearrange("(t p) c -> p t c", p=P),
                    )
                    add_dep_helper(hn_rd.ins, cc.ins, reason="read gathered HT")
                    ps_o = [
                        ps_pool.tile([P, CW], f32, tag="ps",
                                     name=f"ps_o{sfx}_{nu}_{m}")
                        for m in range(NM)
                    ]
                    # corner contribution first: it only needs resident
                    # tiles, so it can run before the gather lands
                    for m in range(NM):
                        nc.tensor.matmul(
                            ps_o[m],
                            lhsT=gtc_sb[:, m * P:(m + 1) * P],
                            rhs=btc_sb[:, cs],
                            start=True,
                            stop=False,
                        )
                    for t in range(NT):
                        for m in range(NM):
                            nc.tensor.matmul(
                                ps_o[m],
                                lhsT=gt_sb[t][:, m * P:(m + 1) * P],
                                rhs=hn_t[:, t * CW:(t + 1) * CW],
                                start=False,
                                stop=(t == NT - 1),
                            )
                    obw_t = obw_pool.tile([P, NM * CW], bf16, tag="obw",
                                          name=f"obw_sb{sfx}_{nu}")
                    for m in range(NM):
                        nc.vector.tensor_copy(
                            obw_t[:, m * CW:(m + 1) * CW], ps_o[m])
                    nc.sync.dma_start(
                        out3[:, :, cs],
                        obw_t.rearrange("p (m c) -> p m c", c=CW),
                    )

    nc.compile()
    return nc


def _get_program():
    if "nc" not in _COMPILED:
        _COMPILED["nc"] = _build_program()
    return _COMPILED["nc"]


def _build_in_maps(A, B):
    import ml_dtypes

    A = np.asarray(A, dtype=np.float32)
    B = np.asarray(B, dtype=np.float32)
    assert A.shape == (M_FULL, N), A.shape
    assert B.shape == (N, N), B.shape

    Q, lam, etr, ebl = _build_spectral()
    bf = ml_dtypes.bfloat16

    a_t = np.ascontiguousarray(A.T.astype(bf))              # [4096, 8192]
    b_t = np.ascontiguousarray(B.T.astype(bf))              # [4096, 4096]
    qa = np.ascontiguousarray(Q.astype(bf))                 # [4096, 768]
    qb = np.ascontiguousarray((Q * lam[None, :]).astype(bf))
    ec = np.ascontiguousarray(
        np.concatenate([-etr, -ebl], axis=1).astype(bf))    # [128, 256]
    # packed corner B.T rows: j<64 -> B.T[j] (for Ebl, support j<=53);
    # j>=64 -> B.T[3968+j] (for Etr, support j>=74)
    btc = np.ascontiguousarray(
        np.concatenate([b_t[0:64, :], b_t[N - 64:N, :]], axis=0).astype(bf))

    return [
        {
            "at": np.ascontiguousarray(a_t[:, c * MS:(c + 1) * MS]),
            "btsl": np.ascontiguousarray(b_t[:, c * CW:(c + 1) * CW]),
            "qa": qa,
            "qb": qb,
            "ec": ec,
            "btc": btc,
        }
        for c in range(NCORES)
    ]


def kernel(A, B):
    in_maps = _build_in_maps(A, B)
    nc = _get_program()
    res = run_bass_kernel_spmd(nc, in_maps, core_ids=list(range(NCORES)))
    return np.concatenate(
        [np.asarray(res.results[c]["out"]) for c in range(NCORES)], axis=0
    ).astype(np.float32)
